# revision 4
# baseline (speedup 1.0000x reference)
"""Distributed 2-layer GCN on 8 TRN2 NeuronCores (Bass/Tile).

Reference computation (PyG-style GCNConv, f32):
    e  = embed_table[node_tokens]            # [N, 256]
    x0 = e @ Wn^T + bn                       # [N, 128]
    h1 = Ahat @ (x0 @ w1^T) + b1 ; z1 = relu(h1)
    h2 = Ahat @ (z1 @ w2^T) + b2             # output [N, 128]
  with Ahat = D^-1/2 (A + I) D^-1/2, deg from dst(+self loops).
  Note (Ahat x) @ w^T == Ahat (x @ w^T), so we aggregate first, project after.

Sharding: nodes are partitioned contiguously across the 8 cores (6250 each,
padded to 6272 = 49 tiles of 128). Each core computes x0 for its own nodes
(embedding dma_gather + projection), all-gathers the full feature matrix
z [50176, 128] between layers, then aggregates its own edges (grouped by dst
owner) with per-edge dma_gather of source rows + one-hot matmul segment-sum
in PSUM, projects, and writes its output shard.

dma_gather indices are int16 (max 32767), so every gathered table is split in
two halves addressed separately:
  - node features: z rows [0, 25088) and [25088, 50176)
  - embedding table: rows [0, 25000) and [25000, 50000), each augmented with
    one zero row so out-of-half tokens gather 0 and the halves can be summed.

Host-side preprocessing (numpy) only handles graph structure: degrees, edge
norms, sorting edges by destination tile, padding to 128-edge one-hot chunks,
and packing int16 index arrays. All feature compute runs on the NeuronCores.
"""

import os

import numpy as np

import concourse.bacc as bacc
import concourse.mybir as mybir
import concourse.tile as tile
from concourse.bass_utils import run_bass_kernel_spmd
from concourse.library_config import mlp

# Problem shape (hardcoded per harness contract)
N = 50000
E = 600000
V = 50000
D_IN = 256
D = 128
NCORES = 8

NPC = N // NCORES            # 6250 nodes per core
TPC = (NPC + 127) // 128     # 49 tiles per core
NPAD = TPC * 128             # 6272 padded nodes per core
NTOT = NCORES * NPAD         # 50176 rows in the all-gathered feature matrix
HALF = NTOT // 2             # 25088: int16-addressable split of z rows
VLO = V // 2                 # 25000: embedding-table split
GT = 7                       # tiles per gather group
GMAX = 1024                  # max indices per dma_gather (SWDGE ring cap)
NG = TPC // GT               # 7 groups per core
F32 = mybir.dt.float32
I16 = mybir.dt.int16


def _wrap_idx(idx_linear):
    """[n] -> [128, n/16] int16: position j at [j%16, j//16], replicated x8."""
    n = idx_linear.shape[0]
    assert n % 16 == 0
    w = idx_linear.astype(np.int16).reshape(-1, 16).T
    return np.tile(w, (8, 1))


def _preprocess(node_tokens, edge_index):
    """Build per-core host arrays + the (core-uniform) chunk layout."""
    src = np.asarray(edge_index[0], dtype=np.int64)
    dst = np.asarray(edge_index[1], dtype=np.int64)
    tok = np.asarray(node_tokens, dtype=np.int64)

    deg = (np.bincount(dst, minlength=N) + 1).astype(np.float32)
    dinv = (1.0 / np.sqrt(deg)).astype(np.float32)

    # Real edges + self loops, all described by (core, tile, half, idx16, dstloc, norm)
    loops = np.arange(N, dtype=np.int64)
    a_src = np.concatenate([src, loops])
    a_dst = np.concatenate([dst, loops])
    a_norm = np.concatenate([dinv[src] * dinv[dst], dinv * dinv]).astype(np.float32)

    core = a_dst // NPC
    dloc = a_dst % NPC
    tloc = dloc // 128
    dstloc = (dloc % 128).astype(np.float32)
    src_gid = (a_src // NPC) * NPAD + (a_src % NPC)
    half = (src_gid >= HALF).astype(np.int64)
    idx16 = np.where(half == 0, src_gid, src_gid - HALF)

    # bucket key: (core, tile, half); sort once
    key = (core * TPC + tloc) * 2 + half
    order = np.argsort(key, kind="stable")
    key_s = key[order]
    idx16_s = idx16[order]
    dstloc_s = dstloc[order]
    norm_s = a_norm[order]
    counts = np.bincount(key_s, minlength=NCORES * TPC * 2).reshape(NCORES, TPC, 2)
    starts = np.zeros(NCORES * TPC * 2 + 1, dtype=np.int64)
    np.cumsum(counts.reshape(-1), out=starts[1:])

    # chunks per (tile, half): max over cores (SPMD graph must be identical)
    cnt = np.maximum(1, -(-counts.max(axis=0) // 128))  # [TPC, 2] in chunks

    # linear edge order per core: g0(lo tiles 0..6)(hi tiles 0..6) g1(...)...
    # chunk layout (uniform): list of (group, half, tile, col_within_gather)
    chunk_tile = []          # per global chunk: tile id
    gather_blocks = []       # per (g, half): n_chunks
    for g in range(NG):
        tiles = range(g * GT, (g + 1) * GT)
        for h in (0, 1):
            blk = 0
            for t in tiles:
                for _ in range(cnt[t, h]):
                    chunk_tile.append(t)
                blk += int(cnt[t, h])
            gather_blocks.append(blk)
    tot_chunks = sum(gather_blocks)

    per_core = []
    for c in range(NCORES):
        idx_parts, dl_parts, nm_parts = [], [], []
        for g in range(NG):
            for h in (0, 1):
                for t in range(g * GT, (g + 1) * GT):
                    k = (c * TPC + t) * 2 + h
                    s, e = starts[k], starts[k] + counts[c, t, h]
                    pad = int(cnt[t, h]) * 128 - (e - s)
                    idx_parts.append(idx16_s[s:e])
                    idx_parts.append(np.zeros(pad, np.int64))
                    dl_parts.append(dstloc_s[s:e])
                    dl_parts.append(np.zeros(pad, np.float32))
                    nm_parts.append(norm_s[s:e])
                    nm_parts.append(np.zeros(pad, np.float32))
        idx_lin = np.concatenate(idx_parts)
        dl_lin = np.concatenate(dl_parts).astype(np.float32)
        nm_lin = np.concatenate(nm_parts).astype(np.float32)
        assert idx_lin.shape[0] == tot_chunks * 128

        # gather idx blocks wrapped separately, concatenated along free dim
        blocks = []
        off = 0
        for nchunks in gather_blocks:
            n = nchunks * 128
            blocks.append(_wrap_idx(idx_lin[off : off + n]))
            off += n
        gidx = np.concatenate(blocks, axis=1)

        meta = np.concatenate(
            [dl_lin.reshape(tot_chunks, 128).T, nm_lin.reshape(tot_chunks, 128).T],
            axis=1,
        ).astype(np.float32)  # [128, 2*tot_chunks]

        # embedding gather indices (per 7-tile group, lo then hi)
        tc_ = tok[c * NPC : (c + 1) * NPC]
        tpad = np.concatenate([tc_, np.zeros(NPAD - NPC, np.int64)])
        lo = np.where(tpad < VLO, tpad, VLO)       # VLO = appended zero row
        hi = np.where(tpad >= VLO, tpad - VLO, V - VLO)
        eblocks = []
        for g in range(NG):
            sl = slice(g * GT * 128, (g + 1) * GT * 128)
            eblocks.append(_wrap_idx(lo[sl]))
            eblocks.append(_wrap_idx(hi[sl]))
        eidx = np.concatenate(eblocks, axis=1)

        per_core.append({"gidx": gidx, "meta": meta, "eidx": eidx})

    layout = {
        "gather_blocks": gather_blocks,   # chunks per (g,half) gather
        "chunk_tile": chunk_tile,         # tile id per global chunk
        "cnt": cnt,                       # [TPC, 2] chunks per (tile, half)
        "tot_chunks": tot_chunks,
    }
    return per_core, layout


STAGE = int(os.environ.get("KSTAGE", "4"))


def _build(layout):
    gather_blocks = layout["gather_blocks"]
    cnt = layout["cnt"]
    tot_chunks = layout["tot_chunks"]
    GCOLS = sum(gather_blocks) * 8          # int16 idx cols = chunks*128/16
    ECOLS = NG * 2 * GT * 8                 # embed idx cols

    nc = bacc.Bacc("TRN2", target_bir_lowering=False, debug=False,
                   num_devices=NCORES)

    tab_lo = nc.dram_tensor("tab_lo", [VLO + 1, D_IN], F32, kind="ExternalInput")
    tab_hi = nc.dram_tensor("tab_hi", [V - VLO + 1, D_IN], F32, kind="ExternalInput")
    eidx_d = nc.dram_tensor("eidx", [128, ECOLS], I16, kind="ExternalInput")
    gidx_d = nc.dram_tensor("gidx", [128, GCOLS], I16, kind="ExternalInput")
    meta_d = nc.dram_tensor("meta", [128, 2 * tot_chunks], F32, kind="ExternalInput")
    wn_d = nc.dram_tensor("wn", [128, 2, D], F32, kind="ExternalInput")
    w1t_d = nc.dram_tensor("w1t", [128, D], F32, kind="ExternalInput")
    w2t_d = nc.dram_tensor("w2t", [128, D], F32, kind="ExternalInput")
    bias_d = nc.dram_tensor("bias", [128, 3], F32, kind="ExternalInput")
    iota_d = nc.dram_tensor("iota", [128, 128], F32, kind="ExternalInput")
    ident_d = nc.dram_tensor("ident", [128, 128], F32, kind="ExternalInput")
    out_d = nc.dram_tensor("out", [NPAD, D], F32, kind="ExternalOutput")

    with tile.TileContext(nc) as tc:
        with (
            tc.tile_pool(name="const", bufs=1) as cp,
            tc.tile_pool(name="embg", bufs=2) as embg,
            tc.tile_pool(name="msgs", bufs=2) as msgp,
            tc.tile_pool(name="oh", bufs=4) as ohp,
            tc.tile_pool(name="work", bufs=3) as wk,
            tc.tile_pool(name="stage", bufs=2) as stg,
            tc.tile_pool(name="psA", bufs=2, space="PSUM") as psA,
            tc.tile_pool(name="psB", bufs=2, space="PSUM") as psB,
            tc.tile_pool(name="psC", bufs=2, space="PSUM") as psC,
            tc.tile_pool(name="dram", bufs=1, space="DRAM") as dram,
        ):
            nc.gpsimd.load_library(mlp)

            # ---- resident constants / metadata ----
            eidx_sb = cp.tile([128, ECOLS], I16)
            gidx_sb = cp.tile([128, GCOLS], I16)
            meta_sb = cp.tile([128, 2 * tot_chunks], F32)
            wn_sb = cp.tile([128, 2, D], F32)
            w1t_sb = cp.tile([128, D], F32)
            w2t_sb = cp.tile([128, D], F32)
            bias_sb = cp.tile([128, 3], F32)
            iota_sb = cp.tile([128, 128], F32)
            ident_sb = cp.tile([128, 128], F32)
            nc.sync.dma_start(eidx_sb[:], eidx_d[:])
            nc.sync.dma_start(gidx_sb[:], gidx_d[:])
            nc.sync.dma_start(meta_sb[:], meta_d[:])
            nc.sync.dma_start(wn_sb[:], wn_d[:])
            nc.sync.dma_start(w1t_sb[:], w1t_d[:])
            nc.sync.dma_start(w2t_sb[:], w2t_d[:])
            nc.sync.dma_start(bias_sb[:], bias_d[:])
            nc.sync.dma_start(iota_sb[:], iota_d[:])
            nc.sync.dma_start(ident_sb[:], ident_d[:])

            z0_loc = dram.tile([NPAD, D], F32)
            z1_loc = dram.tile([NPAD, D], F32)
            z0_full = dram.tile([NTOT, D], F32, addr_space="Shared")
            z1_full = dram.tile([NTOT, D], F32, addr_space="Shared")

            # ---- embedding + input projection: z0 = tab[tok] @ Wn^T + bn ----
            for g in range(NG):
                nidx = GT * 128
                e_lo = embg.tile([128, GT, D_IN], F32, name="e_lo", tag="e_lo")
                e_hi = embg.tile([128, GT, D_IN], F32, name="e_hi", tag="e_hi")
                off = g * 2 * GT * 8
                nc.gpsimd.dma_gather(e_lo[:], tab_lo[:],
                                     eidx_sb[:, off : off + GT * 8],
                                     nidx, nidx, D_IN)
                nc.gpsimd.dma_gather(e_hi[:], tab_hi[:],
                                     eidx_sb[:, off + GT * 8 : off + 2 * GT * 8],
                                     nidx, nidx, D_IN)
                staging = stg.tile([128, GT, D], F32, name="stage0", tag="st0")
                for tt in range(GT):
                    # e^T in two 128-row chunks, lo+hi transposes accumulated
                    x0T_ps = psB.tile([128, 128], F32, name="x0T", tag="pB")
                    for kc in range(2):
                        eT_ps = psA.tile([128, 128], F32, name="eT", tag="pA")
                        nc.tensor.matmul(
                            eT_ps[:], e_lo[:, tt, kc * 128 : (kc + 1) * 128],
                            ident_sb[:], is_transpose=True, start=True, stop=False)
                        nc.tensor.matmul(
                            eT_ps[:], e_hi[:, tt, kc * 128 : (kc + 1) * 128],
                            ident_sb[:], is_transpose=True, start=False, stop=True)
                        eT_sb = wk.tile([128, 128], F32, name="eT_sb", tag="eT_sb")
                        nc.vector.tensor_copy(eT_sb[:], eT_ps[:])
                        nc.tensor.matmul(x0T_ps[:], wn_sb[:, kc, :], eT_sb[:],
                                         start=(kc == 0), stop=(kc == 1))
                    x0T_sb = wk.tile([128, 128], F32, name="x0T_sb", tag="x0T_sb")
                    nc.vector.tensor_scalar_add(x0T_sb[:], x0T_ps[:],
                                                bias_sb[:, 0:1])
                    x0_ps = psC.tile([128, 128], F32, name="x0", tag="pC")
                    nc.tensor.matmul(x0_ps[:], x0T_sb[:], ident_sb[:],
                                     is_transpose=True, start=True, stop=True)
                    nc.vector.tensor_copy(staging[:, tt, :], x0_ps[:])
                dst_rows = z0_loc[g * GT * 128 : (g + 1) * GT * 128, :]
                nc.sync.dma_start(
                    dst_rows.rearrange("(t p) f -> p t f", p=128), staging[:])

            if STAGE == 1:
                nc.sync.dma_start(out_d[:], z0_loc[:])
            if STAGE >= 2:
                nc.gpsimd.collective_compute(
                    "AllGather", mybir.AluOpType.bypass,
                    replica_groups=[list(range(NCORES))],
                    ins=[z0_loc.opt()], outs=[z0_full.opt()])
            if STAGE == 2:
                nc.sync.dma_start(out_d[:], z0_loc[:])

            # ---- GCN layers ----
            def gcn_layer(z_full, wt_sb, bias_col, relu, dest):
                gi_off = 0      # idx col offset into gidx_sb
                ck = 0          # global chunk counter
                for g in range(NG):
                    nlo = gather_blocks[2 * g] * 128
                    nhi = gather_blocks[2 * g + 1] * 128
                    m_lo = msgp.tile([128, gather_blocks[2 * g], D], F32,
                                     name="m_lo", tag="m_lo")
                    m_hi = msgp.tile([128, gather_blocks[2 * g + 1], D], F32,
                                     name="m_hi", tag="m_hi")
                    # SWDGE ring caps one gather at ~1024 indices; split.
                    for mt, nn, src_view in ((m_lo, nlo, z_full[0:HALF, :]),
                                             (m_hi, nhi, z_full[HALF:NTOT, :])):
                        for s0 in range(0, nn, GMAX):
                            sn = min(GMAX, nn - s0)
                            nc.gpsimd.dma_gather(
                                mt[:, s0 // 128 : (s0 + sn) // 128, :], src_view,
                                gidx_sb[:, gi_off + s0 // 16 :
                                        gi_off + (s0 + sn) // 16],
                                sn, sn, D)
                        gi_off += nn // 16

                    staging = stg.tile([128, GT, D], F32, name="stage1", tag="st1")
                    # chunk columns per tile of this group: lo block then hi block
                    lo_base, hi_base = ck, ck + gather_blocks[2 * g]
                    lo_col = hi_col = 0
                    for tt in range(GT):
                        t = g * GT + tt
                        nch_lo, nch_hi = int(cnt[t, 0]), int(cnt[t, 1])
                        aggT_ps = psA.tile([128, 128], F32, name="aggT", tag="pA")
                        nmm = nch_lo + nch_hi
                        mi = 0
                        for src_tile, base, col0, nch in (
                            (m_lo, lo_base, lo_col, nch_lo),
                            (m_hi, hi_base, hi_col, nch_hi),
                        ):
                            for k in range(nch):
                                cidx = base + col0 + k
                                oh = ohp.tile([128, 128], F32, name="oh", tag="oh")
                                nc.vector.tensor_scalar(
                                    oh[:], iota_sb[:],
                                    meta_sb[:, cidx : cidx + 1],
                                    meta_sb[:, tot_chunks + cidx : tot_chunks + cidx + 1],
                                    mybir.AluOpType.is_equal,
                                    mybir.AluOpType.mult)
                                nc.tensor.matmul(
                                    aggT_ps[:], src_tile[:, col0 + k, :], oh[:],
                                    start=(mi == 0), stop=(mi == nmm - 1))
                                mi += 1
                        lo_col += nch_lo
                        hi_col += nch_hi
                        aggT_sb = wk.tile([128, 128], F32, name="aggT_sb",
                                          tag="aggT_sb")
                        nc.vector.tensor_copy(aggT_sb[:], aggT_ps[:])
                        yT_ps = psB.tile([128, 128], F32, name="yT", tag="pB")
                        nc.tensor.matmul(yT_ps[:], wt_sb[:], aggT_sb[:],
                                         start=True, stop=True)
                        yT_sb = wk.tile([128, 128], F32, name="yT_sb", tag="yT_sb")
                        if relu:
                            nc.scalar.activation(
                                yT_sb[:], yT_ps[:],
                                mybir.ActivationFunctionType.Relu,
                                bias=bias_col)
                        else:
                            nc.vector.tensor_scalar_add(yT_sb[:], yT_ps[:],
                                                        bias_col)
                        y_ps = psC.tile([128, 128], F32, name="y", tag="pC")
                        nc.tensor.matmul(y_ps[:], yT_sb[:], ident_sb[:],
                                         is_transpose=True, start=True, stop=True)
                        nc.vector.tensor_copy(staging[:, tt, :], y_ps[:])
                    ck += gather_blocks[2 * g] + gather_blocks[2 * g + 1]
                    dst_rows = dest[g * GT * 128 : (g + 1) * GT * 128, :]
                    nc.sync.dma_start(
                        dst_rows.rearrange("(t p) f -> p t f", p=128), staging[:])

            if STAGE >= 3:
                gcn_layer(z0_full, w1t_sb, bias_sb[:, 1:2], True, z1_loc)
            if STAGE == 3:
                nc.sync.dma_start(out_d[:], z1_loc[:])
            if STAGE >= 4:
                nc.gpsimd.collective_compute(
                    "AllGather", mybir.AluOpType.bypass,
                    replica_groups=[list(range(NCORES))],
                    ins=[z1_loc.opt()], outs=[z1_full.opt()])
                gcn_layer(z1_full, w2t_sb, bias_sb[:, 2:3], False, out_d.ap())

    nc.compile()
    return nc


_CACHE = {}


def _run(inputs, trace=False):
    node_tokens = np.asarray(inputs["node_tokens"])
    edge_index = np.asarray(inputs["edge_index"])
    embed_table = np.asarray(inputs["embed_table"], dtype=np.float32)
    Wn = np.asarray(inputs["W_node_w"], dtype=np.float32)
    bn = np.asarray(inputs["W_node_b"], dtype=np.float32)
    w1 = np.asarray(inputs["w1"], dtype=np.float32)
    b1 = np.asarray(inputs["b1"], dtype=np.float32)
    w2 = np.asarray(inputs["w2"], dtype=np.float32)
    b2 = np.asarray(inputs["b2"], dtype=np.float32)

    per_core, layout = _preprocess(node_tokens, edge_index)

    key = "nc"
    if key not in _CACHE:
        _CACHE[key] = _build(layout)
    nc = _CACHE[key]

    tab_lo = np.concatenate([embed_table[:VLO], np.zeros((1, D_IN), np.float32)])
    tab_hi = np.concatenate([embed_table[VLO:], np.zeros((1, D_IN), np.float32)])
    WnT = Wn.T.copy()  # [256, 128]
    wn = WnT.reshape(2, 128, D).transpose(1, 0, 2).copy()
    bias = np.stack([bn, b1, b2], axis=1).astype(np.float32)
    iota = np.tile(np.arange(128, dtype=np.float32)[None, :], (128, 1))
    ident = np.eye(128, dtype=np.float32)

    in_maps = []
    for c in range(NCORES):
        in_maps.append({
            "tab_lo": tab_lo, "tab_hi": tab_hi,
            "eidx": per_core[c]["eidx"],
            "gidx": per_core[c]["gidx"],
            "meta": per_core[c]["meta"],
            "wn": wn, "w1t": w1.T.copy(), "w2t": w2.T.copy(),
            "bias": bias, "iota": iota, "ident": ident,
        })

    res = run_bass_kernel_spmd(nc, in_maps, core_ids=list(range(NCORES)),
                               trace=trace)
    out = np.concatenate([res.results[c]["out"][:NPC] for c in range(NCORES)],
                         axis=0)
    return out.astype(np.float32), res


def kernel(**inputs):
    out, _ = _run(inputs, trace=False)
    return out
